# revision 8
# baseline (speedup 1.0000x reference)
"""MLPConv (3x3 valid conv -> 256 -> 256 MLP with ReLU) on 8 TRN2 cores.

Data-parallel over batch: 4 images per core. The host pre-transposes each
image to [C, H, W] bf16 so the device PE stream is pure matmuls (no
on-device transposes or casts). The conv is 9 PSUM-accumulated matmuls
(one per filter tap) contracting over C=128 on the partition dim; the
moving operand is a 3D access pattern [C, rows, 62] over the [C, 64, 64]
image so only the 62 valid output columns per row are ever computed.
Both stages keep the [F, pixels] transposed layout; stage-1 evacuation
(bias+ReLU) runs on the ACT engine, stage-2 evacuation is split between
ACT and DVE so it keeps pace with the short stage-2 matmul groups.
Output is written bf16 and the host assembles/casts the final buffer.
"""

import numpy as np
import ml_dtypes

import concourse.bass as bass
import concourse.mybir as mybir
import concourse.tile as tile
from concourse.bass_utils import run_bass_kernel_spmd

B, H, W, C = 32, 64, 64, 128
F = 256
N_CORES = 8
IMG = B // N_CORES                  # 4 images per core
OW = 62                             # valid output cols per row
NPX = 62 * 62                       # 3844 valid output pixels per image
ROWS_A = 40                         # image row-split: tile A rows [0, 40)
ROWS_B = 32                         # tile B rows [32, 64)
ROW_B0 = 32

F32 = mybir.dt.float32
BF16 = mybir.dt.bfloat16
RELU = mybir.ActivationFunctionType.Relu
ADD = mybir.AluOpType.add
MAX = mybir.AluOpType.max

# stage blocks: (r0, nrows) covering output rows 0..61
BLOCKS = [(r0, min(8, 62 - r0)) for r0 in range(0, 62, 8)]


def _split_multi_waits(nc):
    """This container's walrus rejects >1 semaphore wait per instruction
    ("Too many sync wait commands"). Move all but the last wait of each
    instruction onto single-wait NoOps right before it on the same engine."""
    n = 0
    for f in nc.m.functions:
        for bb in f.blocks:
            insts = bb.instructions
            if not any(
                i.sync_info is not None and len(i.sync_info.on_wait) > 1
                for i in insts
            ):
                continue
            new_insts = []
            for inst in insts:
                si = inst.sync_info
                if si is not None and len(si.on_wait) > 1:
                    waits = list(si.on_wait)
                    for k, w in enumerate(waits[:-1]):
                        new_insts.append(
                            mybir.InstNoOp(
                                name=f"{inst.name}-wsplit{k}",
                                engine=inst.engine,
                                bass_nofuse=True,
                                sync_info=mybir.SyncInfo(on_wait=[w], on_update=[]),
                            )
                        )
                        n += 1
                    inst.sync_info = mybir.SyncInfo(
                        on_wait=[waits[-1]], on_update=list(si.on_update)
                    )
                new_insts.append(inst)
            bb.instructions = new_insts
    return n


def build_nc():
    nc = bass.Bass("TRN2", target_bir_lowering=False)
    x = nc.dram_tensor("x", [IMG, C, H, W], BF16, kind="ExternalInput").ap()
    w0 = nc.dram_tensor("w0", [C, 9, F], BF16, kind="ExternalInput").ap()
    w1 = nc.dram_tensor("w1", [128, 2, F], BF16, kind="ExternalInput").ap()
    b0 = nc.dram_tensor("b0", [128, 2], F32, kind="ExternalInput").ap()
    b1 = nc.dram_tensor("b1", [128, 2], F32, kind="ExternalInput").ap()
    out = nc.dram_tensor("out", [2, 128, IMG, NPX], BF16, kind="ExternalOutput").ap()

    with tile.TileContext(nc) as tc:
        with (
            tc.tile_pool(name="consts", bufs=1) as consts,
            tc.tile_pool(name="xa", bufs=IMG) as xap,
            tc.tile_pool(name="xb", bufs=IMG) as xbp,
            tc.tile_pool(name="h1", bufs=2) as h1p,
            tc.tile_pool(name="outb", bufs=2) as outp,
            tc.tile_pool(name="ps1", bufs=4, space="PSUM") as ps1,
            tc.tile_pool(name="ps2", bufs=4, space="PSUM") as ps2,
        ):
            # sync (earliest HWDGE queue) carries the start-critical chunks in
            # consumption order: first taps + first rows gate the first matmuls
            w0t = consts.tile([128, 9, F], BF16)
            w1t = consts.tile([128, 2, F], BF16)
            b0t = consts.tile([128, 2], F32)
            b1t = consts.tile([128, 2], F32)
            xts = []
            for i in range(IMG):
                xat = xap.tile([128, ROWS_A, W], BF16, name="xat")
                xbt = xbp.tile([128, ROWS_B, W], BF16, name="xbt")
                xts.append((xat, xbt))

            # PE warm-up: ~8 matmuls on a zeroed scratch tile keep the PE busy
            # during the input-DMA wait so the HAM clock gate opens (1.2 ->
            # 2.4 GHz) before the real stream starts.
            warm = consts.tile([128, 496], BF16)
            nc.vector.memset(warm[:], 0.0)
            for wi in range(8):
                pw = ps2.tile([128, 496], F32, name="ps2t")
                nc.tensor.matmul(
                    pw[:], warm[:, 0:128], warm[:], start=True, stop=True
                )

            # start-critical DMAs split across the two HWDGE queues
            nc.sync.dma_start(xts[0][0][:, 0:10, :], x[0, :, 0:10, :])
            nc.scalar.dma_start(w0t[:, 0:3, :], w0[:, 0:3, :])
            nc.sync.dma_start(w0t[:, 3:9, :], w0[:, 3:9, :])
            nc.scalar.dma_start(b0t[:], b0)
            nc.sync.dma_start(xts[0][0][:, 10:ROWS_A, :], x[0, :, 10:ROWS_A, :])
            nc.sync.dma_start(xts[0][1][:], x[0, :, ROW_B0:H, :])
            for i in range(1, IMG):
                nc.sync.dma_start(xts[i][0][:], x[i, :, 0:ROWS_A, :])
                nc.sync.dma_start(xts[i][1][:], x[i, :, ROW_B0:H, :])
            nc.scalar.dma_start(w1t[:], w1)
            nc.scalar.dma_start(b1t[:], b1)

            def stage1(i, h1t):
                xat, xbt = xts[i]
                for r0, nr in BLOCKS:
                    npix = nr * OW
                    p0 = r0 * OW
                    # tile A covers input rows [0, 40); B covers [32, 64)
                    xt, base = (xat, 0) if r0 + nr + 1 < ROWS_A else (xbt, ROW_B0)
                    for h in range(2):
                        ps = ps1.tile([128, 496], F32, name="ps1t")
                        for t in range(9):
                            dy, dx = t // 3, t % 3
                            r = r0 + dy - base
                            nc.tensor.matmul(
                                ps[:, :npix],
                                w0t[:, t, 128 * h : 128 * (h + 1)],
                                xt[:, r : r + nr, dx : dx + OW],
                                start=(t == 0),
                                stop=(t == 8),
                            )
                        nc.scalar.activation(
                            h1t[:, h, p0 : p0 + npix],
                            ps[:, :npix],
                            RELU,
                            bias=b0t[:, h : h + 1],
                        )

            def stage2(i, h1t):
                ot = outp.tile([128, 2, NPX], BF16, name="outt")
                # last image streams outputs out per block-pair so the tail
                # after the final matmul is one small store, not a half-image
                qsplits = (
                    [(0, 992), (992, 1984), (1984, 2976), (2976, 3472), (3472, NPX)]
                    if i == IMG - 1
                    else [(0, 1922), (1922, NPX)]
                )
                qi = 0
                for bi, (r0, nr) in enumerate(BLOCKS):
                    npix = nr * OW
                    p0 = r0 * OW
                    for ho in range(2):
                        ps = ps2.tile([128, 496], F32, name="ps2t")
                        for k in range(2):
                            nc.tensor.matmul(
                                ps[:, :npix],
                                w1t[:, k, 128 * ho : 128 * (ho + 1)],
                                h1t[:, k, p0 : p0 + npix],
                                start=(k == 0),
                                stop=(k == 1),
                            )
                        if (2 * bi + ho) % 2 == 0:
                            nc.scalar.activation(
                                ot[:, ho, p0 : p0 + npix],
                                ps[:, :npix],
                                RELU,
                                bias=b1t[:, ho : ho + 1],
                            )
                        else:
                            nc.vector.tensor_scalar(
                                ot[:, ho, p0 : p0 + npix],
                                ps[:, :npix],
                                b1t[:, ho : ho + 1],
                                0.0,
                                ADD,
                                MAX,
                            )
                    # flush any output ranges fully evacuated by now
                    done = p0 + npix
                    while qi < len(qsplits) and qsplits[qi][1] <= done:
                        lo, hi = qsplits[qi]
                        for ho in range(2):
                            eng = nc.gpsimd if (qi + ho) % 2 == 0 else nc.sync
                            eng.dma_start(out[ho, :, i, lo:hi], ot[:, ho, lo:hi])
                        qi += 1

            for i in range(IMG):
                h1t = h1p.tile([128, 2, NPX], BF16, name="h1t")
                stage1(i, h1t)
                stage2(i, h1t)

    _split_multi_waits(nc)
    return nc


_NC_CACHE = None


def kernel(inputs, w0, b0, w1, b1):
    global _NC_CACHE
    bf16 = ml_dtypes.bfloat16
    x = np.asarray(inputs, dtype=np.float32)
    # [B, H, W, C] -> [B, C, H, W] bf16, contiguous
    xt = np.ascontiguousarray(x.transpose(0, 3, 1, 2)).astype(bf16)
    w0b = np.ascontiguousarray(
        np.asarray(w0, np.float32).reshape(9, 128, F).transpose(1, 0, 2)
    ).astype(bf16)
    w1b = np.ascontiguousarray(
        np.asarray(w1, np.float32).reshape(2, 128, F).transpose(1, 0, 2)
    ).astype(bf16)
    b0s = np.ascontiguousarray(np.asarray(b0, np.float32).reshape(2, 128).T)
    b1s = np.ascontiguousarray(np.asarray(b1, np.float32).reshape(2, 128).T)

    if _NC_CACHE is None:
        _NC_CACHE = build_nc()
    nc = _NC_CACHE

    in_maps = [
        {
            "x": xt[c * IMG : (c + 1) * IMG],
            "w0": w0b,
            "w1": w1b,
            "b0": b0s,
            "b1": b1s,
        }
        for c in range(N_CORES)
    ]
    res = run_bass_kernel_spmd(nc, in_maps, core_ids=list(range(N_CORES)))

    final = np.empty((B, 62, 62, F), np.float32)
    vf = final.reshape(F, NPX, B)  # the [F, N, B] view the reference reshapes
    for c in range(N_CORES):
        oc = res.results[c]["out"].reshape(F, IMG, NPX)
        for i in range(IMG):
            vf[:, :, c * IMG + i] = oc[:, i]
    return final


# revision 10
# speedup vs baseline: 1.0157x; 1.0157x over previous
"""MLPConv (3x3 valid conv -> 256 -> 256 MLP with ReLU) on 8 TRN2 cores.

Data-parallel over batch: 4 images per core. The host pre-transposes each
image to [C, H, W] bf16 so the device PE stream is pure matmuls (no
on-device transposes or casts). The conv is 9 PSUM-accumulated matmuls
(one per filter tap) contracting over C=128 on the partition dim; the
moving operand is a 3D access pattern [C, rows, 62] over the [C, 64, 64]
image so only the 62 valid output columns per row are ever computed.
Both stages keep the [F, pixels] transposed layout; stage-1 evacuation
(bias+ReLU) runs on the ACT engine, stage-2 evacuation is split between
ACT and DVE so it keeps pace with the short stage-2 matmul groups.
Output is written bf16 and the host assembles/casts the final buffer.
"""

import numpy as np
import ml_dtypes

import concourse.bass as bass
import concourse.mybir as mybir
import concourse.tile as tile
from concourse.bass_utils import run_bass_kernel_spmd

B, H, W, C = 32, 64, 64, 128
F = 256
N_CORES = 8
IMG = B // N_CORES                  # 4 images per core
OW = 62                             # valid output cols per row
NPX = 62 * 62                       # 3844 valid output pixels per image
ROWS_A = 40                         # image row-split: tile A rows [0, 40)
ROWS_B = 32                         # tile B rows [32, 64)
ROW_B0 = 32

F32 = mybir.dt.float32
BF16 = mybir.dt.bfloat16
RELU = mybir.ActivationFunctionType.Relu
ADD = mybir.AluOpType.add
MAX = mybir.AluOpType.max

# stage blocks: (r0, nrows) covering output rows 0..61
BLOCKS = [(r0, min(8, 62 - r0)) for r0 in range(0, 62, 8)]


def _split_multi_waits(nc):
    """This container's walrus rejects >1 semaphore wait per instruction
    ("Too many sync wait commands"). Move all but the last wait of each
    instruction onto single-wait NoOps right before it on the same engine."""
    n = 0
    for f in nc.m.functions:
        for bb in f.blocks:
            insts = bb.instructions
            if not any(
                i.sync_info is not None and len(i.sync_info.on_wait) > 1
                for i in insts
            ):
                continue
            new_insts = []
            for inst in insts:
                si = inst.sync_info
                if si is not None and len(si.on_wait) > 1:
                    waits = list(si.on_wait)
                    for k, w in enumerate(waits[:-1]):
                        new_insts.append(
                            mybir.InstNoOp(
                                name=f"{inst.name}-wsplit{k}",
                                engine=inst.engine,
                                bass_nofuse=True,
                                sync_info=mybir.SyncInfo(on_wait=[w], on_update=[]),
                            )
                        )
                        n += 1
                    inst.sync_info = mybir.SyncInfo(
                        on_wait=[waits[-1]], on_update=list(si.on_update)
                    )
                new_insts.append(inst)
            bb.instructions = new_insts
    return n


def build_nc():
    nc = bass.Bass("TRN2", target_bir_lowering=False)
    x = nc.dram_tensor("x", [IMG, C, H, W], BF16, kind="ExternalInput").ap()
    w0 = nc.dram_tensor("w0", [C, 9, F], BF16, kind="ExternalInput").ap()
    w1 = nc.dram_tensor("w1", [128, 2, F], BF16, kind="ExternalInput").ap()
    b0 = nc.dram_tensor("b0", [128, 2], F32, kind="ExternalInput").ap()
    b1 = nc.dram_tensor("b1", [128, 2], F32, kind="ExternalInput").ap()
    out = nc.dram_tensor("out", [2, 128, IMG, NPX], BF16, kind="ExternalOutput").ap()

    with tile.TileContext(nc) as tc:
        with (
            tc.tile_pool(name="consts", bufs=1) as consts,
            tc.tile_pool(name="xa", bufs=IMG) as xap,
            tc.tile_pool(name="xb", bufs=IMG) as xbp,
            tc.tile_pool(name="h1", bufs=2) as h1p,
            tc.tile_pool(name="outb", bufs=2) as outp,
            tc.tile_pool(name="ps1", bufs=4, space="PSUM") as ps1,
            tc.tile_pool(name="ps2", bufs=4, space="PSUM") as ps2,
        ):
            # sync (earliest HWDGE queue) carries the start-critical chunks in
            # consumption order: first taps + first rows gate the first matmuls
            w0t = consts.tile([128, 9, F], BF16)
            w1t = consts.tile([128, 2, F], BF16)
            b0t = consts.tile([128, 2], F32)
            b1t = consts.tile([128, 2], F32)
            xts = []
            for i in range(IMG):
                xat = xap.tile([128, ROWS_A, W], BF16, name="xat")
                xbt = xbp.tile([128, ROWS_B, W], BF16, name="xbt")
                xts.append((xat, xbt))

            # PE warm-up: ~8 matmuls on a zeroed scratch tile keep the PE busy
            # during the input-DMA wait so the HAM clock gate opens (1.2 ->
            # 2.4 GHz) before the real stream starts.
            warm = consts.tile([128, 496], BF16)
            nc.vector.memset(warm[:], 0.0)
            for wi in range(10):
                pw = ps2.tile([128, 496], F32, name="ps2t")
                nc.tensor.matmul(
                    pw[:], warm[:, 0:128], warm[:], start=True, stop=True
                )

            # start-critical DMAs: weights on the gpsimd queue in parallel
            # with the first input rows on sync
            nc.gpsimd.dma_start(w0t[:], w0)
            nc.sync.dma_start(xts[0][0][:, 0:10, :], x[0, :, 0:10, :])
            nc.sync.dma_start(b0t[:], b0)
            nc.sync.dma_start(xts[0][0][:, 10:ROWS_A, :], x[0, :, 10:ROWS_A, :])
            nc.sync.dma_start(xts[0][1][:], x[0, :, ROW_B0:H, :])
            for i in range(1, IMG):
                nc.sync.dma_start(xts[i][0][:], x[i, :, 0:ROWS_A, :])
                nc.sync.dma_start(xts[i][1][:], x[i, :, ROW_B0:H, :])
            nc.scalar.dma_start(w1t[:], w1)
            nc.scalar.dma_start(b1t[:], b1)

            def stage1(i, h1t):
                xat, xbt = xts[i]
                for r0, nr in BLOCKS:
                    npix = nr * OW
                    p0 = r0 * OW
                    # tile A covers input rows [0, 40); B covers [32, 64)
                    xt, base = (xat, 0) if r0 + nr + 1 < ROWS_A else (xbt, ROW_B0)
                    for h in range(2):
                        ps = ps1.tile([128, 496], F32, name="ps1t")
                        for t in range(9):
                            dy, dx = t // 3, t % 3
                            r = r0 + dy - base
                            nc.tensor.matmul(
                                ps[:, :npix],
                                w0t[:, t, 128 * h : 128 * (h + 1)],
                                xt[:, r : r + nr, dx : dx + OW],
                                start=(t == 0),
                                stop=(t == 8),
                            )
                        nc.scalar.activation(
                            h1t[:, h, p0 : p0 + npix],
                            ps[:, :npix],
                            RELU,
                            bias=b0t[:, h : h + 1],
                        )

            def stage2(i, h1t):
                ot = outp.tile([128, 2, NPX], BF16, name="outt")
                # last image streams outputs out per block-pair so the tail
                # after the final matmul is one small store, not a half-image
                qsplits = (
                    [(0, 1922), (1922, 2976), (2976, 3472), (3472, NPX)]
                    if i == IMG - 1
                    else [(0, 1922), (1922, NPX)]
                )
                qi = 0
                for bi, (r0, nr) in enumerate(BLOCKS):
                    npix = nr * OW
                    p0 = r0 * OW
                    for ho in range(2):
                        ps = ps2.tile([128, 496], F32, name="ps2t")
                        for k in range(2):
                            nc.tensor.matmul(
                                ps[:, :npix],
                                w1t[:, k, 128 * ho : 128 * (ho + 1)],
                                h1t[:, k, p0 : p0 + npix],
                                start=(k == 0),
                                stop=(k == 1),
                            )
                        if (2 * bi + ho) % 2 == 0:
                            nc.scalar.activation(
                                ot[:, ho, p0 : p0 + npix],
                                ps[:, :npix],
                                RELU,
                                bias=b1t[:, ho : ho + 1],
                            )
                        else:
                            nc.vector.tensor_scalar(
                                ot[:, ho, p0 : p0 + npix],
                                ps[:, :npix],
                                b1t[:, ho : ho + 1],
                                0.0,
                                ADD,
                                MAX,
                            )
                    # flush any output ranges fully evacuated by now
                    done = p0 + npix
                    while qi < len(qsplits) and qsplits[qi][1] <= done:
                        lo, hi = qsplits[qi]
                        for ho in range(2):
                            eng = nc.gpsimd if (qi + ho) % 2 == 0 else nc.sync
                            eng.dma_start(out[ho, :, i, lo:hi], ot[:, ho, lo:hi])
                        qi += 1

            for i in range(IMG):
                h1t = h1p.tile([128, 2, NPX], BF16, name="h1t")
                stage1(i, h1t)
                stage2(i, h1t)

    _split_multi_waits(nc)
    return nc


_NC_CACHE = None


def kernel(inputs, w0, b0, w1, b1):
    global _NC_CACHE
    bf16 = ml_dtypes.bfloat16
    x = np.asarray(inputs, dtype=np.float32)
    # [B, H, W, C] -> [B, C, H, W] bf16, contiguous
    xt = np.ascontiguousarray(x.transpose(0, 3, 1, 2)).astype(bf16)
    w0b = np.ascontiguousarray(
        np.asarray(w0, np.float32).reshape(9, 128, F).transpose(1, 0, 2)
    ).astype(bf16)
    w1b = np.ascontiguousarray(
        np.asarray(w1, np.float32).reshape(2, 128, F).transpose(1, 0, 2)
    ).astype(bf16)
    b0s = np.ascontiguousarray(np.asarray(b0, np.float32).reshape(2, 128).T)
    b1s = np.ascontiguousarray(np.asarray(b1, np.float32).reshape(2, 128).T)

    if _NC_CACHE is None:
        _NC_CACHE = build_nc()
    nc = _NC_CACHE

    in_maps = [
        {
            "x": xt[c * IMG : (c + 1) * IMG],
            "w0": w0b,
            "w1": w1b,
            "b0": b0s,
            "b1": b1s,
        }
        for c in range(N_CORES)
    ]
    res = run_bass_kernel_spmd(nc, in_maps, core_ids=list(range(N_CORES)))

    final = np.empty((B, 62, 62, F), np.float32)
    vf = final.reshape(F, NPX, B)  # the [F, N, B] view the reference reshapes
    for c in range(N_CORES):
        oc = res.results[c]["out"].reshape(F, IMG, NPX)
        for i in range(IMG):
            vf[:, :, c * IMG + i] = oc[:, i]
    return final


# revision 13
# speedup vs baseline: 1.0231x; 1.0073x over previous
"""MLPConv (3x3 valid conv -> 256 -> 256 MLP with ReLU) on 8 TRN2 cores.

Data-parallel over batch: 4 images per core. The host pre-transposes each
image to [C, H, W] bf16 so the device PE stream is pure matmuls (no
on-device transposes or casts). The conv is 9 PSUM-accumulated matmuls
(one per filter tap) contracting over C=128 on the partition dim; the
moving operand is a 3D access pattern [C, rows, 62] over the [C, 64, 64]
image so only the 62 valid output columns per row are ever computed.
Both stages keep the [F, pixels] transposed layout; stage-1 evacuation
(bias+ReLU) runs on the ACT engine, stage-2 evacuation is split between
ACT and DVE so it keeps pace with the short stage-2 matmul groups.
Output is written bf16 and the host assembles/casts the final buffer.
"""

import numpy as np
import ml_dtypes

import concourse.bass as bass
import concourse.mybir as mybir
import concourse.tile as tile
from concourse.bass_utils import run_bass_kernel_spmd

B, H, W, C = 32, 64, 64, 128
F = 256
N_CORES = 8
IMG = B // N_CORES                  # 4 images per core
OW = 62                             # valid output cols per row
NPX = 62 * 62                       # 3844 valid output pixels per image
ROWS_A = 40                         # image row-split: tile A rows [0, 40)
ROWS_B = 32                         # tile B rows [32, 64)
ROW_B0 = 32

F32 = mybir.dt.float32
BF16 = mybir.dt.bfloat16
RELU = mybir.ActivationFunctionType.Relu
ADD = mybir.AluOpType.add
MAX = mybir.AluOpType.max

# stage blocks: (r0, nrows) covering output rows 0..61
BLOCKS = [(r0, min(8, 62 - r0)) for r0 in range(0, 62, 8)]


def _split_multi_waits(nc):
    """This container's walrus rejects >1 semaphore wait per instruction
    ("Too many sync wait commands"). Move all but the last wait of each
    instruction onto single-wait NoOps right before it on the same engine."""
    n = 0
    for f in nc.m.functions:
        for bb in f.blocks:
            insts = bb.instructions
            if not any(
                i.sync_info is not None and len(i.sync_info.on_wait) > 1
                for i in insts
            ):
                continue
            new_insts = []
            for inst in insts:
                si = inst.sync_info
                if si is not None and len(si.on_wait) > 1:
                    waits = list(si.on_wait)
                    for k, w in enumerate(waits[:-1]):
                        new_insts.append(
                            mybir.InstNoOp(
                                name=f"{inst.name}-wsplit{k}",
                                engine=inst.engine,
                                bass_nofuse=True,
                                sync_info=mybir.SyncInfo(on_wait=[w], on_update=[]),
                            )
                        )
                        n += 1
                    inst.sync_info = mybir.SyncInfo(
                        on_wait=[waits[-1]], on_update=list(si.on_update)
                    )
                new_insts.append(inst)
            bb.instructions = new_insts
    return n


def build_nc():
    nc = bass.Bass("TRN2", target_bir_lowering=False)
    x = nc.dram_tensor("x", [IMG, C, H, W], BF16, kind="ExternalInput").ap()
    w0 = nc.dram_tensor("w0", [C, 9, F], BF16, kind="ExternalInput").ap()
    w1 = nc.dram_tensor("w1", [128, 2, F], BF16, kind="ExternalInput").ap()
    b0 = nc.dram_tensor("b0", [128, 2], F32, kind="ExternalInput").ap()
    b1 = nc.dram_tensor("b1", [128, 2], F32, kind="ExternalInput").ap()
    out = nc.dram_tensor("out", [2, 128, IMG, NPX], BF16, kind="ExternalOutput").ap()

    with tile.TileContext(nc) as tc:
        with (
            tc.tile_pool(name="consts", bufs=1) as consts,
            tc.tile_pool(name="xa", bufs=IMG) as xap,
            tc.tile_pool(name="xb", bufs=IMG) as xbp,
            tc.tile_pool(name="h1", bufs=2) as h1p,
            tc.tile_pool(name="outb", bufs=2) as outp,
            tc.tile_pool(name="ps1", bufs=4, space="PSUM") as ps1,
            tc.tile_pool(name="ps2", bufs=4, space="PSUM") as ps2,
        ):
            # sync (earliest HWDGE queue) carries the start-critical chunks in
            # consumption order: first taps + first rows gate the first matmuls
            w0t = consts.tile([128, 9, F], BF16)
            w1t = consts.tile([128, 2, F], BF16)
            b0t = consts.tile([128, 2], F32)
            b1t = consts.tile([128, 2], F32)
            xts = []
            for i in range(IMG):
                xat = xap.tile([128, ROWS_A, W], BF16, name="xat")
                xbt = xbp.tile([128, ROWS_B, W], BF16, name="xbt")
                xts.append((xat, xbt))

            # PE warm-up: ~8 matmuls on a zeroed scratch tile keep the PE busy
            # during the input-DMA wait so the HAM clock gate opens (1.2 ->
            # 2.4 GHz) before the real stream starts.
            warm = consts.tile([128, 496], BF16)
            nc.vector.memset(warm[:], 0.0)
            for wi in range(8):
                pw = ps2.tile([128, 496], F32, name="ps2t")
                nc.tensor.matmul(
                    pw[:], warm[:, 0:128], warm[:], start=True, stop=True
                )

            # start-critical DMAs: first taps on the gpsimd queue in parallel
            # with the first input rows on sync
            nc.gpsimd.dma_start(w0t[:, 0:3, :], w0[:, 0:3, :])
            nc.sync.dma_start(xts[0][0][:, 0:10, :], x[0, :, 0:10, :])
            nc.sync.dma_start(w0t[:, 3:9, :], w0[:, 3:9, :])
            nc.sync.dma_start(b0t[:], b0)
            nc.sync.dma_start(xts[0][0][:, 10:ROWS_A, :], x[0, :, 10:ROWS_A, :])
            nc.sync.dma_start(xts[0][1][:], x[0, :, ROW_B0:H, :])
            for i in range(1, IMG):
                nc.sync.dma_start(xts[i][0][:], x[i, :, 0:ROWS_A, :])
                nc.sync.dma_start(xts[i][1][:], x[i, :, ROW_B0:H, :])
            nc.scalar.dma_start(w1t[:], w1)
            nc.scalar.dma_start(b1t[:], b1)

            def stage1(i, h1t):
                xat, xbt = xts[i]
                for r0, nr in BLOCKS:
                    npix = nr * OW
                    p0 = r0 * OW
                    # tile A covers input rows [0, 40); B covers [32, 64)
                    xt, base = (xat, 0) if r0 + nr + 1 < ROWS_A else (xbt, ROW_B0)
                    for h in range(2):
                        ps = ps1.tile([128, 496], F32, name="ps1t")
                        for t in range(9):
                            dy, dx = t // 3, t % 3
                            r = r0 + dy - base
                            nc.tensor.matmul(
                                ps[:, :npix],
                                w0t[:, t, 128 * h : 128 * (h + 1)],
                                xt[:, r : r + nr, dx : dx + OW],
                                start=(t == 0),
                                stop=(t == 8),
                            )
                        nc.scalar.activation(
                            h1t[:, h, p0 : p0 + npix],
                            ps[:, :npix],
                            RELU,
                            bias=b0t[:, h : h + 1],
                        )

            # dram view [f, h, i, n]: one store covers both output halves
            outr = out.rearrange("h f i n -> f h i n")

            def stage2(i, h1t):
                ot = outp.tile([128, 2, NPX], BF16, name="outt")
                # last image streams outputs out in finer chunks so the tail
                # after the final matmul is one small store, not a half-image
                if i == IMG - 1:
                    qsplits = [(0, 1922), (1922, 2976), (2976, 3472), (3472, NPX)]
                    qengs = [nc.gpsimd, nc.sync, nc.gpsimd, nc.sync]
                else:
                    qsplits = [(0, 1922), (1922, NPX)]
                    qengs = [nc.gpsimd, nc.sync]
                qi = 0
                for bi, (r0, nr) in enumerate(BLOCKS):
                    npix = nr * OW
                    p0 = r0 * OW
                    for ho in range(2):
                        ps = ps2.tile([128, 496], F32, name="ps2t")
                        for k in range(2):
                            nc.tensor.matmul(
                                ps[:, :npix],
                                w1t[:, k, 128 * ho : 128 * (ho + 1)],
                                h1t[:, k, p0 : p0 + npix],
                                start=(k == 0),
                                stop=(k == 1),
                            )
                        if (2 * bi + ho) % 2 == 0:
                            nc.scalar.activation(
                                ot[:, ho, p0 : p0 + npix],
                                ps[:, :npix],
                                RELU,
                                bias=b1t[:, ho : ho + 1],
                            )
                        else:
                            nc.vector.tensor_scalar(
                                ot[:, ho, p0 : p0 + npix],
                                ps[:, :npix],
                                b1t[:, ho : ho + 1],
                                0.0,
                                ADD,
                                MAX,
                            )
                    # flush any output ranges fully evacuated by now
                    done = p0 + npix
                    while qi < len(qsplits) and qsplits[qi][1] <= done:
                        lo, hi = qsplits[qi]
                        qengs[qi].dma_start(outr[:, :, i, lo:hi], ot[:, :, lo:hi])
                        qi += 1

            for i in range(IMG):
                h1t = h1p.tile([128, 2, NPX], BF16, name="h1t")
                stage1(i, h1t)
                stage2(i, h1t)

    _split_multi_waits(nc)
    return nc


_NC_CACHE = None


def kernel(inputs, w0, b0, w1, b1):
    global _NC_CACHE
    bf16 = ml_dtypes.bfloat16
    x = np.asarray(inputs, dtype=np.float32)
    # [B, H, W, C] -> [B, C, H, W] bf16, contiguous
    xt = np.ascontiguousarray(x.transpose(0, 3, 1, 2)).astype(bf16)
    w0b = np.ascontiguousarray(
        np.asarray(w0, np.float32).reshape(9, 128, F).transpose(1, 0, 2)
    ).astype(bf16)
    w1b = np.ascontiguousarray(
        np.asarray(w1, np.float32).reshape(2, 128, F).transpose(1, 0, 2)
    ).astype(bf16)
    b0s = np.ascontiguousarray(np.asarray(b0, np.float32).reshape(2, 128).T)
    b1s = np.ascontiguousarray(np.asarray(b1, np.float32).reshape(2, 128).T)

    if _NC_CACHE is None:
        _NC_CACHE = build_nc()
    nc = _NC_CACHE

    in_maps = [
        {
            "x": xt[c * IMG : (c + 1) * IMG],
            "w0": w0b,
            "w1": w1b,
            "b0": b0s,
            "b1": b1s,
        }
        for c in range(N_CORES)
    ]
    res = run_bass_kernel_spmd(nc, in_maps, core_ids=list(range(N_CORES)))

    final = np.empty((B, 62, 62, F), np.float32)
    vf = final.reshape(F, NPX, B)  # the [F, N, B] view the reference reshapes
    for c in range(N_CORES):
        oc = res.results[c]["out"].reshape(F, IMG, NPX)
        for i in range(IMG):
            vf[:, :, c * IMG + i] = oc[:, i]
    return final
